# revision 10
# baseline (speedup 1.0000x reference)
"""Causal single-head attention forward on 8 Trainium2 NeuronCores.

Problem: x:(512,256,384) f32, Wq/Wk/Wv:(384,64) f32
  k = x@Wk; q = x@Wq; v = x@Wv
  wei = softmax(mask(q k^T / sqrt(384)))
  out = wei @ v                       -> (512, 256, 64) f32

Strategy: pure data parallel over batch (64 batches/core, no collectives).
Per core, batches are processed in macro-iterations of 2 batches:
  - load x naturally [T-part, C-free] (contiguous DMA), cast f32->bf16
  - transpose on PE (identity matmul) to xT [C-part, T-free]
  - packed [Wq|Wk] projection -> qkT [2H=128 part, T]  (one matmul chain)
  - v projection direct -> v [T-part, H]
  - S computed TRANSPOSED: sT[k, q] = kT.T @ qT so that exp(sT) = pT is
    directly the lhsT of the PV matmul (no P transpose needed).
  - causal: block (k1, q0) never computed; diagonal blocks masked by a 0/1
    upper-triangular mult after exp (gpsimd).
  - softmax denominators via a ones-column appended to v (col 64 of the PV
    matmul accumulates row sums); final scale = DVE reciprocal + per-partition
    tensor_scalar multiply.
All matmuls bf16 inputs with fp32 PSUM accumulation.
"""

import os
from contextlib import ExitStack

import numpy as np

import concourse.bass as bass
import concourse.mybir as mybir
import concourse.tile as tile
from concourse import bacc
from concourse.bass_utils import run_bass_kernel_spmd
from concourse.masks import make_identity

B, T, C, H = 512, 256, 384, 64
N_CORES = 8
B_LOC = B // N_CORES          # 64 batches per core
MACRO = 2                     # batches per macro-iteration
N_MACRO = B_LOC // MACRO      # 32
NC_CHUNKS = C // 128          # 3
SCALE = float(C) ** -0.5

F32 = mybir.dt.float32
BF16 = mybir.dt.bfloat16


def build_attention_kernel(ctx, tc, out_d, x_d, wq_d, wk_d, wv_d):
    nc = tc.nc
    pool = lambda *a, **k: ctx.enter_context(tc.tile_pool(*a, **k))

    # ---------------- constants / weights (once) ----------------
    cpool = pool(name="const", bufs=1)
    ident = cpool.tile([128, 128], BF16, tag="ident")
    make_identity(nc, ident[:, :])

    # upper-triangular (incl diag) 0/1 mask in bf16: mask[j, i] = 1 if i >= j
    maskT = cpool.tile([128, 128], BF16, tag="maskT")
    nc.gpsimd.memset(maskT[:, :], 1.0)
    nc.gpsimd.affine_select(
        out=maskT[:, :],
        in_=maskT[:, :],
        compare_op=mybir.AluOpType.is_ge,
        fill=0.0,
        base=0,
        pattern=[[1, 128]],       # iota = -j + i  (channel j, free i)
        channel_multiplier=-1,
    )

    # weights: DRAM (384,64) -> staging f32 [128, 3, 64] -> packed bf16
    wst = cpool.tile([128, 3 * NC_CHUNKS, 64], F32, tag="wstage")
    for i, wd in enumerate((wq_d, wk_d, wv_d)):
        nc.sync.dma_start(
            wst[:, i * NC_CHUNKS:(i + 1) * NC_CHUNKS, :],
            wd.rearrange("(n p) h -> p n h", p=128),
        )
    # bf16 weights [128(C within chunk), chunk, 64]; Wq pre-scaled by C^-0.5
    wq = cpool.tile([128, NC_CHUNKS, 64], BF16, tag="wq")
    nc.vector.tensor_scalar_mul(wq[:, :, :], wst[:, 0:NC_CHUNKS, :], SCALE)
    wk = cpool.tile([128, NC_CHUNKS, 64], BF16, tag="wk")
    nc.vector.tensor_copy(wk[:, :, :], wst[:, NC_CHUNKS:2 * NC_CHUNKS, :])
    wv = cpool.tile([128, NC_CHUNKS, 64], BF16, tag="wv")
    nc.vector.tensor_copy(wv[:, :, :], wst[:, 2 * NC_CHUNKS:3 * NC_CHUNKS, :])

    # ---------------- pools ----------------
    xf_pool = pool(name="xf", bufs=3)       # f32 natural x
    xb_pool = pool(name="xb", bufs=3)       # bf16 natural x
    xts_pool = pool(name="xts", bufs=6)     # bf16 xT in SBUF (3/macro)
    qs_pool = pool(name="qs", bufs=2)       # bf16 qT in SBUF [64, 512]
    ks_pool = pool(name="ks", bufs=2)       # bf16 kT in SBUF [64, 512]
    vs_pool = pool(name="vs", bufs=2)       # bf16 v (+ones col)
    pt_pool = pool(name="pt", bufs=3)       # bf16 pT
    os_pool = pool(name="os", bufs=3)       # f32 out staging
    rec_pool = pool(name="rec", bufs=3)     # f32 reciprocal

    xtp_pool = pool(name="xtp", bufs=2, space="PSUM")   # xT psum
    qp_pool = pool(name="qp", bufs=1, space="PSUM")     # qT psum [64, 512]
    kp_pool = pool(name="kp", bufs=1, space="PSUM")     # kT psum [64, 512]
    vp_pool = pool(name="vp", bufs=1, space="PSUM")     # v psum
    stp_pool = pool(name="stp", bufs=1, space="PSUM")   # sT psum
    op_pool = pool(name="op", bufs=2, space="PSUM")     # out psum

    x_r = x_d.rearrange("b (n p) c -> b n p c", p=128)      # [B_LOC, 2, 128, C]
    out_r = out_d.rearrange("b (n p) h -> b p n h", p=128)  # [B_LOC, 128, 2, H]

    for it in range(N_MACRO):
        b0 = it * MACRO

        # ---- load x (2 batches) and cast to bf16 ----
        xf = xf_pool.tile([128, MACRO, 2, C], F32, tag="xf")
        for b in range(MACRO):
            for t in range(2):
                nc.sync.dma_start(xf[:, b, t, :], x_r[b0 + b, t, :, :])
        xb = xb_pool.tile([128, MACRO, 2, C], BF16, tag="xb")
        nc.gpsimd.tensor_copy(xb[:, :, :, :], xf[:, :, :, :])

        # ---- transpose x on PE: xT[c][p=c-chunk, col = b*256 + t*128 + tt] ----
        xts = []
        for c in range(NC_CHUNKS):
            xtp = xtp_pool.tile([128, 512], BF16, tag="xtp")
            for b in range(MACRO):
                for t in range(2):
                    nc.tensor.transpose(
                        xtp[:, (b * 2 + t) * 128:(b * 2 + t) * 128 + 128],
                        xb[:, b, t, c * 128:(c + 1) * 128],
                        ident[:, :],
                    )
            xt = xts_pool.tile([128, 512], BF16, tag="xts")
            nc.scalar.copy(xt[:, :], xtp[:, :])
            xts.append(xt)

        # ---- q/k projections: qT, kT [64(H), 512(T of b0|b1)] ----
        qp = qp_pool.tile([64, 512], F32, tag="qp")
        kp = kp_pool.tile([64, 512], F32, tag="kp")
        for c in range(NC_CHUNKS):
            nc.tensor.matmul(
                qp[:, :], wq[:, c, :], xts[c][:, :],
                start=(c == 0), stop=(c == NC_CHUNKS - 1),
            )
        for c in range(NC_CHUNKS):
            nc.tensor.matmul(
                kp[:, :], wk[:, c, :], xts[c][:, :],
                start=(c == 0), stop=(c == NC_CHUNKS - 1),
            )
        qs = qs_pool.tile([64, 512], BF16, tag="qs")
        nc.vector.tensor_copy(qs[:, :], qp[:, :])
        ks = ks_pool.tile([64, 512], BF16, tag="ks")
        nc.vector.tensor_copy(ks[:, :], kp[:, :])

        # ---- v projection: v [128(T-chunk), 4 chunks * 64] ----
        vp = vp_pool.tile([128, 256], F32, tag="vp")
        for q in range(4):  # q = b*2 + t
            for c in range(NC_CHUNKS):
                nc.tensor.matmul(
                    vp[:, q * 64:(q + 1) * 64],
                    xts[c][:, q * 128:(q + 1) * 128],
                    wv[:, c, :],
                    start=(c == 0), stop=(c == NC_CHUNKS - 1),
                )
        vs = vs_pool.tile([128, 4, 65], BF16, tag="vs")
        nc.vector.tensor_copy(
            vs[:, :, 0:64], vp.rearrange("p (q h) -> p q h", h=64)
        )
        nc.vector.memset(vs[:, :, 64], 1.0)

        # ---- per-batch attention ----
        for b in range(MACRO):
            qcol = b * 256
            # sT: [128, 0:256] = sT(k0, q0|q1); [128, 256:384] = sT(k1, q1)
            stp = stp_pool.tile([128, 384], F32, tag="stp")
            nc.tensor.matmul(
                stp[:, 0:256],
                ks[:, qcol:qcol + 128],            # kT chunk k0 [64, 128]
                qs[:, qcol:qcol + 256],            # qT (both chunks) [64, 256]
                start=True, stop=True,
            )
            nc.tensor.matmul(
                stp[:, 256:384],
                ks[:, qcol + 128:qcol + 256],      # kT chunk k1
                qs[:, qcol + 128:qcol + 256],      # qT chunk q1
                start=True, stop=True,
            )
            # pT = exp(sT)  (no max subtraction: logits are O(+-3))
            pt = pt_pool.tile([128, 384], BF16, tag="pt")
            nc.scalar.activation(
                pt[:, :], stp[:, :], mybir.ActivationFunctionType.Exp
            )
            # causal 0/1 masking of the two diagonal blocks
            nc.gpsimd.tensor_mul(pt[:, 0:128], pt[:, 0:128], maskT[:, :])
            nc.gpsimd.tensor_mul(pt[:, 256:384], pt[:, 256:384], maskT[:, :])

            # out = pT.T @ [v | 1]: cols 0:65 = q0 (k0 only), 65:130 = q1
            op = op_pool.tile([128, 130], F32, tag="op")
            nc.tensor.matmul(
                op[:, 0:65], pt[:, 0:128], vs[:, b * 2 + 0, :],
                start=True, stop=True,
            )
            nc.tensor.matmul(
                op[:, 65:130], pt[:, 128:256], vs[:, b * 2 + 0, :],
                start=True, stop=False,
            )
            nc.tensor.matmul(
                op[:, 65:130], pt[:, 256:384], vs[:, b * 2 + 1, :],
                start=False, stop=True,
            )
            # denominators are cols 64 and 129
            rec = rec_pool.tile([128, 2], F32, tag="rec")
            nc.vector.reciprocal(
                rec[:, :], op.rearrange("p (n q) -> p n q", q=65)[:, :, 64]
            )
            osb = os_pool.tile([128, 2, 64], F32, tag="os")
            nc.vector.tensor_scalar_mul(osb[:, 0, :], op[:, 0:64], rec[:, 0:1])
            nc.vector.tensor_scalar_mul(osb[:, 1, :], op[:, 65:129], rec[:, 1:2])
            nc.sync.dma_start(out_r[b0 + b, :, :, :], osb[:, :, :])


_CACHED = None


def _build():
    global _CACHED
    if _CACHED is not None:
        return _CACHED
    nc = bacc.Bacc(
        "TRN2",
        target_bir_lowering=False,
        debug=False,
        num_devices=N_CORES,
    )
    x_d = nc.dram_tensor("x", [B_LOC, T, C], F32, kind="ExternalInput").ap()
    wq_d = nc.dram_tensor("Wq", [C, H], F32, kind="ExternalInput").ap()
    wk_d = nc.dram_tensor("Wk", [C, H], F32, kind="ExternalInput").ap()
    wv_d = nc.dram_tensor("Wv", [C, H], F32, kind="ExternalInput").ap()
    out_d = nc.dram_tensor("out", [B_LOC, T, H], F32, kind="ExternalOutput").ap()
    with tile.TileContext(nc) as tc, ExitStack() as ctx:
        build_attention_kernel(ctx, tc, out_d, x_d, wq_d, wk_d, wv_d)
    nc.compile()
    _CACHED = nc
    return nc


def kernel(x, Wk, Wq, Wv, _trace=False):
    x = np.ascontiguousarray(np.asarray(x, dtype=np.float32))
    Wk = np.ascontiguousarray(np.asarray(Wk, dtype=np.float32))
    Wq = np.ascontiguousarray(np.asarray(Wq, dtype=np.float32))
    Wv = np.ascontiguousarray(np.asarray(Wv, dtype=np.float32))

    nc = _build()
    xs = x.reshape(N_CORES, B_LOC, T, C)
    in_maps = [
        {"x": xs[i], "Wq": Wq, "Wk": Wk, "Wv": Wv} for i in range(N_CORES)
    ]
    res = run_bass_kernel_spmd(
        nc, in_maps, core_ids=list(range(N_CORES)), trace=_trace
    )
    out = np.concatenate([r["out"] for r in res.results], axis=0)
    if _trace:
        return out, res
    return out


# revision 13
# speedup vs baseline: 253.6735x; 253.6735x over previous
"""Causal single-head attention forward on 8 Trainium2 NeuronCores.

Problem: x:(512,256,384) f32, Wq/Wk/Wv:(384,64) f32
  k = x@Wk; q = x@Wq; v = x@Wv
  wei = softmax(mask(q k^T / sqrt(384)))
  out = wei @ v                       -> (512, 256, 64) f32

Strategy: pure data parallel over batch (64 batches/core, no collectives).
Per core, batches are processed in macro-iterations of 2 batches:
  - load x naturally [T-part, C-free] (contiguous DMA), cast f32->bf16
  - transpose on PE (identity matmul) to xT [C-part, T-free]
  - packed [Wq|Wk] projection -> qkT [2H=128 part, T]  (one matmul chain)
  - v projection direct -> v [T-part, H]
  - S computed TRANSPOSED: sT[k, q] = kT.T @ qT so that exp(sT) = pT is
    directly the lhsT of the PV matmul (no P transpose needed).
  - causal: block (k1, q0) never computed; diagonal blocks masked by a 0/1
    upper-triangular mult after exp (gpsimd).
  - softmax denominators via a ones-column appended to v (col 64 of the PV
    matmul accumulates row sums); final scale = DVE reciprocal + per-partition
    tensor_scalar multiply.
All matmuls bf16 inputs with fp32 PSUM accumulation.
"""

import os
from contextlib import ExitStack

import numpy as np

import concourse.bass as bass
import concourse.mybir as mybir
import concourse.tile as tile
from concourse import bacc
from concourse.bass_utils import run_bass_kernel_spmd
from concourse.masks import make_identity

B, T, C, H = 512, 256, 384, 64
N_CORES = 8
B_LOC = B // N_CORES          # 64 batches per core
MACRO = 2                     # batches per macro-iteration
N_MACRO = B_LOC // MACRO      # 32
NC_CHUNKS = C // 128          # 3
SCALE = float(C) ** -0.5

F32 = mybir.dt.float32
BF16 = mybir.dt.bfloat16


def build_attention_kernel(ctx, tc, out_d, x_d, wq_d, wk_d, wv_d):
    nc = tc.nc
    pool = lambda *a, **k: ctx.enter_context(tc.tile_pool(*a, **k))

    # ---------------- constants / weights (once) ----------------
    cpool = pool(name="const", bufs=1)
    ident = cpool.tile([128, 128], BF16, tag="ident")
    make_identity(nc, ident[:, :])

    # upper-triangular (incl diag) 0/1 mask in bf16: mask[j, i] = 1 if i >= j
    maskT = cpool.tile([128, 128], BF16, tag="maskT")
    nc.gpsimd.memset(maskT[:, :], 1.0)
    nc.gpsimd.affine_select(
        out=maskT[:, :],
        in_=maskT[:, :],
        compare_op=mybir.AluOpType.is_ge,
        fill=0.0,
        base=0,
        pattern=[[1, 128]],       # iota = -j + i  (channel j, free i)
        channel_multiplier=-1,
    )

    # weights: DRAM (384,64) -> staging f32 [128, 3, 64] -> packed bf16
    wst = cpool.tile([128, 3 * NC_CHUNKS, 64], F32, tag="wstage")
    for i, wd in enumerate((wq_d, wk_d, wv_d)):
        nc.sync.dma_start(
            wst[:, i * NC_CHUNKS:(i + 1) * NC_CHUNKS, :],
            wd.rearrange("(n p) h -> p n h", p=128),
        )
    # bf16 weights [128(C within chunk), chunk, 64]; Wq pre-scaled by C^-0.5
    wq = cpool.tile([128, NC_CHUNKS, 64], BF16, tag="wq")
    nc.vector.tensor_scalar_mul(wq[:, :, :], wst[:, 0:NC_CHUNKS, :], SCALE)
    wk = cpool.tile([128, NC_CHUNKS, 64], BF16, tag="wk")
    nc.vector.tensor_copy(wk[:, :, :], wst[:, NC_CHUNKS:2 * NC_CHUNKS, :])
    wv = cpool.tile([128, NC_CHUNKS, 64], BF16, tag="wv")
    nc.vector.tensor_copy(wv[:, :, :], wst[:, 2 * NC_CHUNKS:3 * NC_CHUNKS, :])

    # ---------------- pools ----------------
    xf_pool = pool(name="xf", bufs=3)       # f32 natural x
    xb_pool = pool(name="xb", bufs=3)       # bf16 natural x
    xts_pool = pool(name="xts", bufs=6)     # bf16 xT in SBUF (3/macro)
    qs_pool = pool(name="qs", bufs=2)       # bf16 qT in SBUF [64, 512]
    ks_pool = pool(name="ks", bufs=2)       # bf16 kT in SBUF [64, 512]
    vs_pool = pool(name="vs", bufs=2)       # bf16 v (+ones col)
    pt_pool = pool(name="pt", bufs=3)       # bf16 pT
    os_pool = pool(name="os", bufs=3)       # f32 out staging
    rec_pool = pool(name="rec", bufs=3)     # f32 reciprocal

    xtp_pool = pool(name="xtp", bufs=2, space="PSUM")   # xT psum
    qp_pool = pool(name="qp", bufs=1, space="PSUM")     # qT psum [64, 512]
    kp_pool = pool(name="kp", bufs=1, space="PSUM")     # kT psum [64, 512]
    vp_pool = pool(name="vp", bufs=1, space="PSUM")     # v psum
    stp_pool = pool(name="stp", bufs=1, space="PSUM")   # sT psum
    op_pool = pool(name="op", bufs=2, space="PSUM")     # out psum

    x_r = x_d.rearrange("b (n p) c -> b n p c", p=128)      # [B_LOC, 2, 128, C]
    out_r = out_d.rearrange("b (n p) h -> b p n h", p=128)  # [B_LOC, 128, 2, H]

    for it in range(N_MACRO):
        b0 = it * MACRO

        # ---- load x (2 batches) and cast to bf16 ----
        xf = xf_pool.tile([128, MACRO, 2, C], F32, tag="xf")
        for b in range(MACRO):
            for t in range(2):
                nc.sync.dma_start(xf[:, b, t, :], x_r[b0 + b, t, :, :])
        xb = xb_pool.tile([128, MACRO, 2, C], BF16, tag="xb")
        nc.gpsimd.tensor_copy(xb[:, :, :, :], xf[:, :, :, :])

        # ---- transpose x on PE: xT[c][p=c-chunk, col = b*256 + t*128 + tt] ----
        xts = []
        for c in range(NC_CHUNKS):
            xtp = xtp_pool.tile([128, 512], BF16, tag="xtp")
            for b in range(MACRO):
                for t in range(2):
                    nc.tensor.transpose(
                        xtp[:, (b * 2 + t) * 128:(b * 2 + t) * 128 + 128],
                        xb[:, b, t, c * 128:(c + 1) * 128],
                        ident[:, :],
                    )
            xt = xts_pool.tile([128, 512], BF16, tag="xts")
            nc.scalar.copy(xt[:, :], xtp[:, :])
            xts.append(xt)

        # ---- q/k projections: qT, kT [64(H), 512(T of b0|b1)] ----
        qp = qp_pool.tile([64, 512], F32, tag="qp")
        kp = kp_pool.tile([64, 512], F32, tag="kp")
        for c in range(NC_CHUNKS):
            nc.tensor.matmul(
                qp[:, :], wq[:, c, :], xts[c][:, :],
                start=(c == 0), stop=(c == NC_CHUNKS - 1),
            )
        for c in range(NC_CHUNKS):
            nc.tensor.matmul(
                kp[:, :], wk[:, c, :], xts[c][:, :],
                start=(c == 0), stop=(c == NC_CHUNKS - 1),
            )
        qs = qs_pool.tile([64, 512], BF16, tag="qs")
        nc.vector.tensor_copy(qs[:, :], qp[:, :])
        ks = ks_pool.tile([64, 512], BF16, tag="ks")
        nc.vector.tensor_copy(ks[:, :], kp[:, :])

        # ---- v projection: v [128(T-chunk), 4 chunks * 64] ----
        vp = vp_pool.tile([128, 256], F32, tag="vp")
        for q in range(4):  # q = b*2 + t
            for c in range(NC_CHUNKS):
                nc.tensor.matmul(
                    vp[:, q * 64:(q + 1) * 64],
                    xts[c][:, q * 128:(q + 1) * 128],
                    wv[:, c, :],
                    start=(c == 0), stop=(c == NC_CHUNKS - 1),
                )
        vs = vs_pool.tile([128, 4, 65], BF16, tag="vs")
        nc.vector.tensor_copy(
            vs[:, :, 0:64], vp.rearrange("p (q h) -> p q h", h=64)
        )
        nc.vector.memset(vs[:, :, 64], 1.0)

        # ---- per-batch attention ----
        for b in range(MACRO):
            qcol = b * 256
            # sT: [128, 0:256] = sT(k0, q0|q1); [128, 256:384] = sT(k1, q1)
            stp = stp_pool.tile([128, 384], F32, tag="stp")
            nc.tensor.matmul(
                stp[:, 0:256],
                ks[:, qcol:qcol + 128],            # kT chunk k0 [64, 128]
                qs[:, qcol:qcol + 256],            # qT (both chunks) [64, 256]
                start=True, stop=True,
            )
            nc.tensor.matmul(
                stp[:, 256:384],
                ks[:, qcol + 128:qcol + 256],      # kT chunk k1
                qs[:, qcol + 128:qcol + 256],      # qT chunk q1
                start=True, stop=True,
            )
            # pT = exp(sT)  (no max subtraction: logits are O(+-3))
            pt = pt_pool.tile([128, 384], BF16, tag="pt")
            nc.scalar.activation(
                pt[:, :], stp[:, :], mybir.ActivationFunctionType.Exp
            )
            # causal 0/1 masking of the two diagonal blocks
            nc.gpsimd.tensor_mul(pt[:, 0:128], pt[:, 0:128], maskT[:, :])
            nc.gpsimd.tensor_mul(pt[:, 256:384], pt[:, 256:384], maskT[:, :])

            # out = pT.T @ [v | 1]: cols 0:65 = q0 (k0 only), 65:130 = q1
            op = op_pool.tile([128, 130], F32, tag="op")
            nc.tensor.matmul(
                op[:, 0:65], pt[:, 0:128], vs[:, b * 2 + 0, :],
                start=True, stop=True,
            )
            nc.tensor.matmul(
                op[:, 65:130], pt[:, 128:256], vs[:, b * 2 + 0, :],
                start=True, stop=False,
            )
            nc.tensor.matmul(
                op[:, 65:130], pt[:, 256:384], vs[:, b * 2 + 1, :],
                start=False, stop=True,
            )
            # denominators are cols 64 and 129
            rec = rec_pool.tile([128, 2], F32, tag="rec")
            nc.vector.reciprocal(
                rec[:, :], op.rearrange("p (n q) -> p n q", q=65)[:, :, 64]
            )
            osb = os_pool.tile([128, 2, 64], F32, tag="os")
            nc.vector.tensor_scalar_mul(osb[:, 0, :], op[:, 0:64], rec[:, 0:1])
            nc.vector.tensor_scalar_mul(osb[:, 1, :], op[:, 65:129], rec[:, 1:2])
            nc.sync.dma_start(out_r[b0 + b, :, :, :], osb[:, :, :])


_CACHED = None


def _build():
    global _CACHED
    if _CACHED is not None:
        return _CACHED
    nc = bacc.Bacc(
        "TRN2",
        target_bir_lowering=False,
        debug=False,
        num_devices=N_CORES,
    )
    x_d = nc.dram_tensor("x", [B_LOC, T, C], F32, kind="ExternalInput").ap()
    wq_d = nc.dram_tensor("Wq", [C, H], F32, kind="ExternalInput").ap()
    wk_d = nc.dram_tensor("Wk", [C, H], F32, kind="ExternalInput").ap()
    wv_d = nc.dram_tensor("Wv", [C, H], F32, kind="ExternalInput").ap()
    out_d = nc.dram_tensor("out", [B_LOC, T, H], F32, kind="ExternalOutput").ap()
    with tile.TileContext(nc) as tc, ExitStack() as ctx:
        build_attention_kernel(ctx, tc, out_d, x_d, wq_d, wk_d, wv_d)
    nc.compile()
    _CACHED = nc
    return nc


_RUNNER = None


def _get_runner():
    """Persistent jitted SPMD executor (compiles/loads the NEFF once)."""
    global _RUNNER
    if _RUNNER is not None:
        return _RUNNER

    import jax
    from jax.sharding import Mesh, PartitionSpec
    from jax.experimental.shard_map import shard_map
    from concourse import bass2jax

    nc = _build()
    bass2jax.install_neuronx_cc_hook()

    partition_name = (
        nc.partition_id_tensor.name if nc.partition_id_tensor else None
    )
    in_names, out_names, out_avals = [], [], []
    for alloc in nc.m.functions[0].allocations:
        if not isinstance(alloc, mybir.MemoryLocationSet):
            continue
        name = alloc.memorylocations[0].name
        if alloc.kind == "ExternalInput":
            if name != partition_name:
                in_names.append(name)
        elif alloc.kind == "ExternalOutput":
            out_names.append(name)
            out_avals.append(
                jax.core.ShapedArray(
                    tuple(alloc.tensor_shape), mybir.dt.np(alloc.dtype)
                )
            )
    n_params = len(in_names)
    all_in_names = in_names + out_names
    if partition_name is not None:
        all_in_names = all_in_names + [partition_name]

    def _body(*args):
        operands = list(args)
        if partition_name is not None:
            operands.append(bass2jax.partition_id_tensor())
        outs = bass2jax._bass_exec_p.bind(
            *operands,
            out_avals=tuple(out_avals),
            in_names=tuple(all_in_names),
            out_names=tuple(out_names),
            lowering_input_output_aliases=(),
            sim_require_finite=True,
            sim_require_nnan=True,
            nc=nc,
        )
        return tuple(outs)

    devices = jax.devices()[:N_CORES]
    mesh = Mesh(np.asarray(devices), ("core",))
    fn = jax.jit(
        shard_map(
            _body,
            mesh=mesh,
            in_specs=(PartitionSpec("core"),) * (n_params + len(out_names)),
            out_specs=(PartitionSpec("core"),) * len(out_names),
            check_rep=False,
        ),
        keep_unused=True,
    )
    zero_outs = [
        np.zeros((N_CORES * a.shape[0], *a.shape[1:]), a.dtype) for a in out_avals
    ]
    _RUNNER = (fn, in_names, out_names, out_avals, zero_outs)
    return _RUNNER


def _global_inputs(x, Wk, Wq, Wv):
    """Concatenated per-core inputs keyed by BIR input name."""
    reps = {
        "x": np.ascontiguousarray(x, dtype=np.float32),
        "Wq": np.tile(np.asarray(Wq, np.float32), (N_CORES, 1)),
        "Wk": np.tile(np.asarray(Wk, np.float32), (N_CORES, 1)),
        "Wv": np.tile(np.asarray(Wv, np.float32), (N_CORES, 1)),
    }
    return reps


def kernel(x, Wk, Wq, Wv):
    x = np.asarray(x, dtype=np.float32)
    fn, in_names, out_names, out_avals, zero_outs = _get_runner()
    gi = _global_inputs(x, Wk, Wq, Wv)
    args = [gi[n] for n in in_names] + zero_outs
    outs = fn(*args)
    out = np.asarray(outs[out_names.index("out")])
    return out.astype(np.float32)


# revision 18
# speedup vs baseline: 19732.4502x; 77.7868x over previous
"""Causal single-head attention forward on 8 Trainium2 NeuronCores.

Problem: x:(512,256,384) f32, Wq/Wk/Wv:(384,64) f32
  k = x@Wk; q = x@Wq; v = x@Wv
  wei = softmax(mask(q k^T / sqrt(384)))
  out = wei @ v                       -> (512, 256, 64) f32

Strategy: pure data parallel over batch (64 batches/core, no collectives).
Per core, batches are processed in macro-iterations of 2 batches:
  - load x naturally [T-part, C-free] (contiguous DMA), cast f32->bf16
  - transpose on PE (identity matmul) to xT [C-part, T-free]
  - packed [Wq|Wk] projection -> qkT [2H=128 part, T]  (one matmul chain)
  - v projection direct -> v [T-part, H]
  - S computed TRANSPOSED: sT[k, q] = kT.T @ qT so that exp(sT) = pT is
    directly the lhsT of the PV matmul (no P transpose needed).
  - causal: block (k1, q0) never computed; diagonal blocks masked by a 0/1
    upper-triangular mult after exp (gpsimd).
  - softmax denominators via a ones-column appended to v (col 64 of the PV
    matmul accumulates row sums); final scale = DVE reciprocal + per-partition
    tensor_scalar multiply.
All matmuls bf16 inputs with fp32 PSUM accumulation.
"""

import os
from contextlib import ExitStack

import numpy as np

import concourse.bass as bass
import concourse.mybir as mybir
import concourse.tile as tile
from concourse import bacc
from concourse.bass_utils import run_bass_kernel_spmd
from concourse.masks import make_identity

B, T, C, H = 512, 256, 384, 64
N_CORES = 8
B_LOC = B // N_CORES          # 64 batches per core
MACRO = 2                     # batches per macro-iteration
N_MACRO = B_LOC // MACRO      # 32
NC_CHUNKS = C // 128          # 3
SCALE = float(C) ** -0.5

F32 = mybir.dt.float32
BF16 = mybir.dt.bfloat16


def build_attention_kernel(ctx, tc, out_d, x_d, wq_d, wk_d, wv_d, reps=1):
    nc = tc.nc
    pool = lambda *a, **k: ctx.enter_context(tc.tile_pool(*a, **k))

    # ---------------- constants / weights (once) ----------------
    cpool = pool(name="const", bufs=1)
    ident = cpool.tile([128, 128], BF16, tag="ident")
    make_identity(nc, ident[:, :])

    # upper-triangular (incl diag) 0/1 mask in bf16: mask[j, i] = 1 if i >= j
    maskT = cpool.tile([128, 128], BF16, tag="maskT")
    nc.gpsimd.memset(maskT[:, :], 1.0)
    nc.gpsimd.affine_select(
        out=maskT[:, :],
        in_=maskT[:, :],
        compare_op=mybir.AluOpType.is_ge,
        fill=0.0,
        base=0,
        pattern=[[1, 128]],       # iota = -j + i  (channel j, free i)
        channel_multiplier=-1,
    )

    # weights: DRAM (384,64) -> staging f32 [128, 3, 64] -> packed bf16
    wst = cpool.tile([128, 3 * NC_CHUNKS, 64], F32, tag="wstage")
    for i, wd in enumerate((wq_d, wk_d, wv_d)):
        nc.sync.dma_start(
            wst[:, i * NC_CHUNKS:(i + 1) * NC_CHUNKS, :],
            wd.rearrange("(n p) h -> p n h", p=128),
        )
    # bf16 weights [128(C within chunk), chunk, 64]; Wq pre-scaled by C^-0.5
    wq = cpool.tile([128, NC_CHUNKS, 64], BF16, tag="wq")
    nc.vector.tensor_scalar_mul(wq[:, :, :], wst[:, 0:NC_CHUNKS, :], SCALE)
    wk = cpool.tile([128, NC_CHUNKS, 64], BF16, tag="wk")
    nc.vector.tensor_copy(wk[:, :, :], wst[:, NC_CHUNKS:2 * NC_CHUNKS, :])
    wv = cpool.tile([128, NC_CHUNKS, 64], BF16, tag="wv")
    nc.vector.tensor_copy(wv[:, :, :], wst[:, 2 * NC_CHUNKS:3 * NC_CHUNKS, :])

    # ---------------- pools ----------------
    xf_pool = pool(name="xf", bufs=3)       # f32 natural x
    xb_pool = pool(name="xb", bufs=3)       # bf16 natural x
    xts_pool = pool(name="xts", bufs=6)     # bf16 xT in SBUF (3/macro)
    qs_pool = pool(name="qs", bufs=2)       # bf16 qT in SBUF [64, 512]
    ks_pool = pool(name="ks", bufs=2)       # bf16 kT in SBUF [64, 512]
    vs_pool = pool(name="vs", bufs=2)       # bf16 v (+ones col)
    pt_pool = pool(name="pt", bufs=3)       # bf16 pT
    os_pool = pool(name="os", bufs=3)       # f32 out staging
    rec_pool = pool(name="rec", bufs=3)     # f32 reciprocal

    xtp_pool = pool(name="xtp", bufs=2, space="PSUM")   # xT psum
    qp_pool = pool(name="qp", bufs=1, space="PSUM")     # qT psum [64, 512]
    kp_pool = pool(name="kp", bufs=1, space="PSUM")     # kT psum [64, 512]
    vp_pool = pool(name="vp", bufs=1, space="PSUM")     # v psum
    stp_pool = pool(name="stp", bufs=1, space="PSUM")   # sT psum
    op_pool = pool(name="op", bufs=2, space="PSUM")     # out psum

    x_r = x_d.rearrange("b (n p) c -> b n p c", p=128)      # [B_LOC, 2, 128, C]
    out_r = out_d.rearrange("b (n p) h -> b p n h", p=128)  # [B_LOC, 128, 2, H]

    if reps > 1:
        rep_ctx = tc.For_i(0, reps, 1)
        rep_ctx.__enter__()

    for it in range(N_MACRO):
        b0 = it * MACRO

        # ---- load x (2 batches) and cast to bf16 ----
        xf = xf_pool.tile([128, MACRO, 2, C], F32, tag="xf")
        for b in range(MACRO):
            for t in range(2):
                nc.sync.dma_start(xf[:, b, t, :], x_r[b0 + b, t, :, :])
        xb = xb_pool.tile([128, MACRO, 2, C], BF16, tag="xb")
        nc.gpsimd.tensor_copy(xb[:, :, :, :], xf[:, :, :, :])

        # ---- transpose x on PE: xT[c][p=c-chunk, col = b*256 + t*128 + tt] ----
        xts = []
        for c in range(NC_CHUNKS):
            xtp = xtp_pool.tile([128, 512], BF16, tag="xtp")
            for b in range(MACRO):
                for t in range(2):
                    nc.tensor.transpose(
                        xtp[:, (b * 2 + t) * 128:(b * 2 + t) * 128 + 128],
                        xb[:, b, t, c * 128:(c + 1) * 128],
                        ident[:, :],
                    )
            xt = xts_pool.tile([128, 512], BF16, tag="xts")
            nc.scalar.copy(xt[:, :], xtp[:, :])
            xts.append(xt)

        # ---- q/k projections: qT, kT [64(H), 512(T of b0|b1)] ----
        qp = qp_pool.tile([64, 512], F32, tag="qp")
        kp = kp_pool.tile([64, 512], F32, tag="kp")
        for c in range(NC_CHUNKS):
            nc.tensor.matmul(
                qp[:, :], wq[:, c, :], xts[c][:, :],
                start=(c == 0), stop=(c == NC_CHUNKS - 1),
            )
        for c in range(NC_CHUNKS):
            nc.tensor.matmul(
                kp[:, :], wk[:, c, :], xts[c][:, :],
                start=(c == 0), stop=(c == NC_CHUNKS - 1),
            )
        qs = qs_pool.tile([64, 512], BF16, tag="qs")
        nc.vector.tensor_copy(qs[:, :], qp[:, :])
        ks = ks_pool.tile([64, 512], BF16, tag="ks")
        nc.vector.tensor_copy(ks[:, :], kp[:, :])

        # ---- v projection: v [128(T-chunk), 4 chunks * 64] ----
        vp = vp_pool.tile([128, 256], F32, tag="vp")
        for q in range(4):  # q = b*2 + t
            for c in range(NC_CHUNKS):
                nc.tensor.matmul(
                    vp[:, q * 64:(q + 1) * 64],
                    xts[c][:, q * 128:(q + 1) * 128],
                    wv[:, c, :],
                    start=(c == 0), stop=(c == NC_CHUNKS - 1),
                )
        vs = vs_pool.tile([128, 4, 65], BF16, tag="vs")
        nc.vector.tensor_copy(
            vs[:, :, 0:64], vp.rearrange("p (q h) -> p q h", h=64)
        )
        nc.vector.memset(vs[:, :, 64], 1.0)

        # ---- per-batch attention ----
        for b in range(MACRO):
            qcol = b * 256
            # sT: [128, 0:256] = sT(k0, q0|q1); [128, 256:384] = sT(k1, q1)
            stp = stp_pool.tile([128, 384], F32, tag="stp")
            nc.tensor.matmul(
                stp[:, 0:256],
                ks[:, qcol:qcol + 128],            # kT chunk k0 [64, 128]
                qs[:, qcol:qcol + 256],            # qT (both chunks) [64, 256]
                start=True, stop=True,
            )
            nc.tensor.matmul(
                stp[:, 256:384],
                ks[:, qcol + 128:qcol + 256],      # kT chunk k1
                qs[:, qcol + 128:qcol + 256],      # qT chunk q1
                start=True, stop=True,
            )
            # pT = exp(sT)  (no max subtraction: logits are O(+-3))
            pt = pt_pool.tile([128, 384], BF16, tag="pt")
            nc.scalar.activation(
                pt[:, :], stp[:, :], mybir.ActivationFunctionType.Exp
            )
            # causal 0/1 masking of the two diagonal blocks
            nc.gpsimd.tensor_mul(pt[:, 0:128], pt[:, 0:128], maskT[:, :])
            nc.gpsimd.tensor_mul(pt[:, 256:384], pt[:, 256:384], maskT[:, :])

            # out = pT.T @ [v | 1]: cols 0:65 = q0 (k0 only), 65:130 = q1
            op = op_pool.tile([128, 130], F32, tag="op")
            nc.tensor.matmul(
                op[:, 0:65], pt[:, 0:128], vs[:, b * 2 + 0, :],
                start=True, stop=True,
            )
            nc.tensor.matmul(
                op[:, 65:130], pt[:, 128:256], vs[:, b * 2 + 0, :],
                start=True, stop=False,
            )
            nc.tensor.matmul(
                op[:, 65:130], pt[:, 256:384], vs[:, b * 2 + 1, :],
                start=False, stop=True,
            )
            # denominators are cols 64 and 129
            rec = rec_pool.tile([128, 2], F32, tag="rec")
            nc.vector.reciprocal(
                rec[:, :], op.rearrange("p (n q) -> p n q", q=65)[:, :, 64]
            )
            osb = os_pool.tile([128, 2, 64], F32, tag="os")
            nc.vector.tensor_scalar_mul(osb[:, 0, :], op[:, 0:64], rec[:, 0:1])
            nc.vector.tensor_scalar_mul(osb[:, 1, :], op[:, 65:129], rec[:, 1:2])
            nc.sync.dma_start(out_r[b0 + b, :, :, :], osb[:, :, :])

    if reps > 1:
        rep_ctx.__exit__(None, None, None)


_CACHED = {}


def _build(reps=1):
    if reps in _CACHED:
        return _CACHED[reps]
    nc = bacc.Bacc(
        "TRN2",
        target_bir_lowering=False,
        debug=False,
        num_devices=N_CORES,
    )
    x_d = nc.dram_tensor("x", [B_LOC, T, C], F32, kind="ExternalInput").ap()
    wq_d = nc.dram_tensor("Wq", [C, H], F32, kind="ExternalInput").ap()
    wk_d = nc.dram_tensor("Wk", [C, H], F32, kind="ExternalInput").ap()
    wv_d = nc.dram_tensor("Wv", [C, H], F32, kind="ExternalInput").ap()
    out_d = nc.dram_tensor("out", [B_LOC, T, H], F32, kind="ExternalOutput").ap()
    with tile.TileContext(nc) as tc, ExitStack() as ctx:
        build_attention_kernel(ctx, tc, out_d, x_d, wq_d, wk_d, wv_d, reps=reps)
    nc.compile()
    _CACHED[reps] = nc
    return nc


_RUNNER = {}


def _get_runner(reps=1):
    """Persistent jitted SPMD executor (compiles/loads the NEFF once)."""
    if reps in _RUNNER:
        return _RUNNER[reps]

    import jax
    from jax.sharding import Mesh, PartitionSpec
    from jax.experimental.shard_map import shard_map
    from concourse import bass2jax

    nc = _build(reps)
    bass2jax.install_neuronx_cc_hook()

    partition_name = (
        nc.partition_id_tensor.name if nc.partition_id_tensor else None
    )
    in_names, out_names, out_avals = [], [], []
    for alloc in nc.m.functions[0].allocations:
        if not isinstance(alloc, mybir.MemoryLocationSet):
            continue
        name = alloc.memorylocations[0].name
        if alloc.kind == "ExternalInput":
            if name != partition_name:
                in_names.append(name)
        elif alloc.kind == "ExternalOutput":
            out_names.append(name)
            out_avals.append(
                jax.core.ShapedArray(
                    tuple(alloc.tensor_shape), mybir.dt.np(alloc.dtype)
                )
            )
    n_params = len(in_names)
    all_in_names = in_names + out_names
    if partition_name is not None:
        all_in_names = all_in_names + [partition_name]

    def _body(*args):
        operands = list(args)
        if partition_name is not None:
            operands.append(bass2jax.partition_id_tensor())
        outs = bass2jax._bass_exec_p.bind(
            *operands,
            out_avals=tuple(out_avals),
            in_names=tuple(all_in_names),
            out_names=tuple(out_names),
            lowering_input_output_aliases=(),
            sim_require_finite=True,
            sim_require_nnan=True,
            nc=nc,
        )
        return tuple(outs)

    devices = jax.devices()[:N_CORES]
    mesh = Mesh(np.asarray(devices), ("core",))
    fn = jax.jit(
        shard_map(
            _body,
            mesh=mesh,
            in_specs=(PartitionSpec("core"),) * (n_params + len(out_names)),
            out_specs=(PartitionSpec("core"),) * len(out_names),
            check_rep=False,
        ),
        keep_unused=True,
    )
    zero_outs = [
        np.zeros((N_CORES * a.shape[0], *a.shape[1:]), a.dtype) for a in out_avals
    ]
    _RUNNER[reps] = (fn, in_names, out_names, out_avals, zero_outs)
    return _RUNNER[reps]


def _global_inputs(x, Wk, Wq, Wv):
    """Concatenated per-core inputs keyed by BIR input name."""
    reps = {
        "x": np.ascontiguousarray(x, dtype=np.float32),
        "Wq": np.tile(np.asarray(Wq, np.float32), (N_CORES, 1)),
        "Wk": np.tile(np.asarray(Wk, np.float32), (N_CORES, 1)),
        "Wv": np.tile(np.asarray(Wv, np.float32), (N_CORES, 1)),
    }
    return reps


def kernel(x, Wk, Wq, Wv):
    x = np.asarray(x, dtype=np.float32)
    fn, in_names, out_names, out_avals, zero_outs = _get_runner()
    gi = _global_inputs(x, Wk, Wq, Wv)
    args = [gi[n] for n in in_names] + zero_outs
    outs = fn(*args)
    out = np.asarray(outs[out_names.index("out")])
    return out.astype(np.float32)


# revision 31
# speedup vs baseline: 20370.5186x; 1.0323x over previous
"""Causal single-head attention forward on 8 Trainium2 NeuronCores.

Problem: x:(512,256,384) f32, Wq/Wk/Wv:(384,64) f32
  k = x@Wk; q = x@Wq; v = x@Wv
  wei = softmax(mask(q k^T / sqrt(384)))
  out = wei @ v                       -> (512, 256, 64) f32

Strategy: pure data parallel over batch (64 batches/core, no collectives).
Per core, batches are processed in macro-iterations of 2 batches:
  - load x naturally [T-part, C-free] (contiguous DMA), cast f32->bf16
  - transpose on PE (identity matmul) to xT [C-part, T-free]
  - packed [Wq|Wk] projection -> qkT [2H=128 part, T]  (one matmul chain)
  - v projection direct -> v [T-part, H]
  - S computed TRANSPOSED: sT[k, q] = kT.T @ qT so that exp(sT) = pT is
    directly the lhsT of the PV matmul (no P transpose needed).
  - causal: block (k1, q0) never computed; diagonal blocks masked by a 0/1
    upper-triangular mult after exp (gpsimd).
  - softmax denominators via a ones-column appended to v (col 64 of the PV
    matmul accumulates row sums); final scale = DVE reciprocal + per-partition
    tensor_scalar multiply.
All matmuls bf16 inputs with fp32 PSUM accumulation.
"""

import os
from contextlib import ExitStack

import numpy as np

import concourse.bass as bass
import concourse.mybir as mybir
import concourse.tile as tile
from concourse import bacc
from concourse.bass_utils import run_bass_kernel_spmd
from concourse.masks import make_identity

B, T, C, H = 512, 256, 384, 64
N_CORES = 8
B_LOC = B // N_CORES          # 64 batches per core
MACRO = 2                     # batches per macro-iteration
N_MACRO = B_LOC // MACRO      # 32
NC_CHUNKS = C // 128          # 3
SCALE = float(C) ** -0.5

F32 = mybir.dt.float32
F32R = mybir.dt.float32r
BF16 = mybir.dt.bfloat16


def build_attention_kernel(ctx, tc, out_d, x_d, wq_d, wk_d, wv_d, reps=1):
    nc = tc.nc
    pool = lambda *a, **k: ctx.enter_context(tc.tile_pool(*a, **k))

    # ---------------- constants / weights (once) ----------------
    cpool = pool(name="const", bufs=1)
    ident = cpool.tile([128, 128], BF16, tag="ident")
    make_identity(nc, ident[:, :])
    # f32r identity: gpsimd/DVE can't produce f32r, so stage f32 identity
    # through DRAM and DMA it back into an f32r-typed tile.
    ident_stage = cpool.tile([128, 128], F32, tag="identstage")
    make_identity(nc, ident_stage[:, :])
    ident_dram = nc.dram_tensor("ident_dram", [128, 128], F32).ap()
    nc.sync.dma_start(ident_dram[:, :], ident_stage[:, :])
    ident_f = cpool.tile([128, 128], F32R, tag="identf")
    nc.sync.dma_start(ident_f[:, :], ident_dram.bitcast(F32R))
    ident_f32r = ident_f[:, :]

    # upper-triangular (incl diag) 0/1 mask in bf16: mask[j, i] = 1 if i >= j
    maskT = cpool.tile([128, 128], BF16, tag="maskT")
    nc.gpsimd.memset(maskT[:, :], 1.0)
    nc.gpsimd.affine_select(
        out=maskT[:, :],
        in_=maskT[:, :],
        compare_op=mybir.AluOpType.is_ge,
        fill=0.0,
        base=0,
        pattern=[[1, 128]],       # iota = -j + i  (channel j, free i)
        channel_multiplier=-1,
    )

    # weights: DRAM (384,64) -> staging f32 [128, 3, 64] -> packed bf16
    wst = cpool.tile([128, 3 * NC_CHUNKS, 64], F32, tag="wstage")
    for i, wd in enumerate((wq_d, wk_d, wv_d)):
        nc.sync.dma_start(
            wst[:, i * NC_CHUNKS:(i + 1) * NC_CHUNKS, :],
            wd.rearrange("(n p) h -> p n h", p=128),
        )
    # bf16 weights [128(C within chunk), chunk, *]; Wq pre-scaled by C^-0.5.
    # wkv packs [Wk | Wv] along the output dim -> kv projection in one chain.
    wq = cpool.tile([128, NC_CHUNKS, 64], BF16, tag="wq")
    nc.vector.tensor_scalar_mul(wq[:, :, :], wst[:, 0:NC_CHUNKS, :], SCALE)
    wkv = cpool.tile([128, NC_CHUNKS, 128], BF16, tag="wkv")
    nc.vector.tensor_copy(wkv[:, :, 0:64], wst[:, NC_CHUNKS:2 * NC_CHUNKS, :])
    nc.vector.tensor_copy(wkv[:, :, 64:128], wst[:, 2 * NC_CHUNKS:3 * NC_CHUNKS, :])

    # ---------------- pools ----------------
    xf_pool = pool(name="xf", bufs=3)       # f32 natural x
    xts_pool = pool(name="xts", bufs=6)     # bf16 xT in SBUF (3/macro)
    qs_pool = pool(name="qs", bufs=2)       # bf16 qT in SBUF [64, 512]
    kvs_pool = pool(name="kvs", bufs=2)     # bf16 [kT|vT] in SBUF [128, 512]
    vs_pool = pool(name="vs", bufs=2)       # bf16 v (+ones col)
    pt_pool = pool(name="pt", bufs=3)       # bf16 pT
    os_pool = pool(name="os", bufs=3)       # f32 out staging
    rec_pool = pool(name="rec", bufs=3)     # f32 reciprocal

    xtp_pool = pool(name="xtp", bufs=2, space="PSUM")   # xT psum
    qp_pool = pool(name="qp", bufs=1, space="PSUM")     # qT psum [64, 512]
    kvp_pool = pool(name="kvp", bufs=1, space="PSUM")   # kvT psum [128, 512]
    vtp_pool = pool(name="vtp", bufs=1, space="PSUM")   # v-natural psum
    stp_pool = pool(name="stp", bufs=2, space="PSUM")   # sT psum
    op_pool = pool(name="op", bufs=1, space="PSUM")     # out psum

    x_r = x_d.rearrange("b (n p) c -> b n p c", p=128)      # [B_LOC, 2, 128, C]
    out_r = out_d.rearrange("b (n p) h -> b p n h", p=128)  # [B_LOC, 128, 2, H]

    if reps > 1:
        rep_ctx = tc.For_i(0, reps, 1)
        rep_ctx.__enter__()

    for it in range(N_MACRO):
        b0 = it * MACRO

        # ---- load x (2 batches), alternate the two HWDGE rings ----
        xf = xf_pool.tile([128, MACRO, 2, C], F32R, tag="xf")
        for b in range(MACRO):
            for t in range(2):
                eng = nc.sync if (b * 2 + t) % 2 == 0 else nc.scalar
                eng.dma_start(
                    xf[:, b, t, :], x_r[b0 + b, t, :, :].bitcast(F32R)
                )

        # ---- transpose x on PE (f32r in, bf16 out): ----
        # xT[c][p = C within chunk, col = b*256 + t*128 + tt]
        xts = []
        for c in range(NC_CHUNKS):
            xtp = xtp_pool.tile([128, 512], F32R, tag="xtp")
            for b in range(MACRO):
                for t in range(2):
                    nc.tensor.transpose(
                        xtp[:, (b * 2 + t) * 128:(b * 2 + t) * 128 + 128],
                        xf[:, b, t, c * 128:(c + 1) * 128],
                        ident_f32r,
                    )
            xt = xts_pool.tile([128, 512], BF16, tag="xts")
            nc.scalar.copy(xt[:, :], xtp[:, :].bitcast(F32))
            xts.append(xt)

        # ---- projections: qT [64, 512]; [kT|vT] packed [128, 512] ----
        qp = qp_pool.tile([64, 512], F32, tag="qp")
        kvp = kvp_pool.tile([128, 512], F32, tag="kvp")
        for c in range(NC_CHUNKS):
            nc.tensor.matmul(
                qp[:, :], wq[:, c, :], xts[c][:, :],
                start=(c == 0), stop=(c == NC_CHUNKS - 1),
            )
        for c in range(NC_CHUNKS):
            nc.tensor.matmul(
                kvp[:, :], wkv[:, c, :], xts[c][:, :],
                start=(c == 0), stop=(c == NC_CHUNKS - 1),
            )
        qs = qs_pool.tile([64, 512], BF16, tag="qs")
        nc.vector.tensor_copy(qs[:, :], qp[:, :])
        kvs = kvs_pool.tile([128, 512], BF16, tag="kvs")
        nc.vector.tensor_copy(kvs[:, :], kvp[:, :])

        # ---- v natural [128(T), 64] via PE transpose of the vT half ----
        vtp = vtp_pool.tile([128, 256], BF16, tag="vtp")
        for q in range(4):  # q = b*2 + t
            nc.tensor.transpose(
                vtp[:, q * 64:(q + 1) * 64],
                kvs[64:128, q * 128:(q + 1) * 128],
                ident[64:128, 64:128],
            )
        vs = vs_pool.tile([128, 4, 65], BF16, tag="vs")
        nc.vector.tensor_copy(
            vs[:, :, 0:64], vtp.rearrange("p (q h) -> p q h", h=64)
        )
        nc.vector.memset(vs[:, :, 64], 1.0)

        # ---- per-batch attention ----
        for b in range(MACRO):
            qcol = b * 256
            # sT: [128, 0:256] = sT(k0, q0|q1); [128, 256:384] = sT(k1, q1)
            stp = stp_pool.tile([128, 384], F32, tag="stp")
            nc.tensor.matmul(
                stp[:, 0:256],
                kvs[0:64, qcol:qcol + 128],        # kT chunk k0 [64, 128]
                qs[:, qcol:qcol + 256],            # qT (both chunks) [64, 256]
                start=True, stop=True,
            )
            nc.tensor.matmul(
                stp[:, 256:384],
                kvs[0:64, qcol + 128:qcol + 256],  # kT chunk k1
                qs[:, qcol + 128:qcol + 256],      # qT chunk q1
                start=True, stop=True,
            )
            # pT = exp(sT)  (no max subtraction: logits are O(+-3))
            pt = pt_pool.tile([128, 384], BF16, tag="pt")
            nc.scalar.activation(
                pt[:, :], stp[:, :], mybir.ActivationFunctionType.Exp
            )
            # causal 0/1 masking of the two diagonal blocks
            nc.gpsimd.tensor_mul(pt[:, 0:128], pt[:, 0:128], maskT[:, :])
            nc.gpsimd.tensor_mul(pt[:, 256:384], pt[:, 256:384], maskT[:, :])

            # out = pT.T @ [v | 1]: cols 0:65 = q0 (k0 only), 65:130 = q1
            op = op_pool.tile([128, 130], F32, tag="op")
            nc.tensor.matmul(
                op[:, 0:65], pt[:, 0:128], vs[:, b * 2 + 0, :],
                start=True, stop=True,
            )
            nc.tensor.matmul(
                op[:, 65:130], pt[:, 128:256], vs[:, b * 2 + 0, :],
                start=True, stop=False,
            )
            nc.tensor.matmul(
                op[:, 65:130], pt[:, 256:384], vs[:, b * 2 + 1, :],
                start=False, stop=True,
            )
            # denominators are cols 64 and 129
            rec = rec_pool.tile([128, 2], F32, tag="rec")
            nc.vector.reciprocal(
                rec[:, :], op.rearrange("p (n q) -> p n q", q=65)[:, :, 64]
            )
            osb = os_pool.tile([128, 2, 64], F32, tag="os")
            nc.vector.tensor_scalar_mul(osb[:, 0, :], op[:, 0:64], rec[:, 0:1])
            nc.vector.tensor_scalar_mul(osb[:, 1, :], op[:, 65:129], rec[:, 1:2])
            eng = nc.sync if b % 2 == 0 else nc.scalar
            eng.dma_start(out_r[b0 + b, :, :, :], osb[:, :, :])

    if reps > 1:
        rep_ctx.__exit__(None, None, None)


_CACHED = {}


def _build(reps=1):
    if reps in _CACHED:
        return _CACHED[reps]
    nc = bacc.Bacc(
        "TRN2",
        target_bir_lowering=False,
        debug=False,
        num_devices=N_CORES,
    )
    x_d = nc.dram_tensor("x", [B_LOC, T, C], F32, kind="ExternalInput").ap()
    wq_d = nc.dram_tensor("Wq", [C, H], F32, kind="ExternalInput").ap()
    wk_d = nc.dram_tensor("Wk", [C, H], F32, kind="ExternalInput").ap()
    wv_d = nc.dram_tensor("Wv", [C, H], F32, kind="ExternalInput").ap()
    out_d = nc.dram_tensor("out", [B_LOC, T, H], F32, kind="ExternalOutput").ap()
    with tile.TileContext(nc) as tc, ExitStack() as ctx:
        build_attention_kernel(ctx, tc, out_d, x_d, wq_d, wk_d, wv_d, reps=reps)
    nc.compile()
    _CACHED[reps] = nc
    return nc


_RUNNER = {}


def _get_runner(reps=1):
    """Persistent jitted SPMD executor (compiles/loads the NEFF once)."""
    if reps in _RUNNER:
        return _RUNNER[reps]

    import jax
    from jax.sharding import Mesh, PartitionSpec
    from jax.experimental.shard_map import shard_map
    from concourse import bass2jax

    nc = _build(reps)
    bass2jax.install_neuronx_cc_hook()

    partition_name = (
        nc.partition_id_tensor.name if nc.partition_id_tensor else None
    )
    in_names, out_names, out_avals = [], [], []
    for alloc in nc.m.functions[0].allocations:
        if not isinstance(alloc, mybir.MemoryLocationSet):
            continue
        name = alloc.memorylocations[0].name
        if alloc.kind == "ExternalInput":
            if name != partition_name:
                in_names.append(name)
        elif alloc.kind == "ExternalOutput":
            out_names.append(name)
            out_avals.append(
                jax.core.ShapedArray(
                    tuple(alloc.tensor_shape), mybir.dt.np(alloc.dtype)
                )
            )
    n_params = len(in_names)
    all_in_names = in_names + out_names
    if partition_name is not None:
        all_in_names = all_in_names + [partition_name]

    def _body(*args):
        operands = list(args)
        if partition_name is not None:
            operands.append(bass2jax.partition_id_tensor())
        outs = bass2jax._bass_exec_p.bind(
            *operands,
            out_avals=tuple(out_avals),
            in_names=tuple(all_in_names),
            out_names=tuple(out_names),
            lowering_input_output_aliases=(),
            sim_require_finite=True,
            sim_require_nnan=True,
            nc=nc,
        )
        return tuple(outs)

    devices = jax.devices()[:N_CORES]
    mesh = Mesh(np.asarray(devices), ("core",))
    fn = jax.jit(
        shard_map(
            _body,
            mesh=mesh,
            in_specs=(PartitionSpec("core"),) * (n_params + len(out_names)),
            out_specs=(PartitionSpec("core"),) * len(out_names),
            check_rep=False,
        ),
        keep_unused=True,
    )
    zero_outs = [
        np.zeros((N_CORES * a.shape[0], *a.shape[1:]), a.dtype) for a in out_avals
    ]
    _RUNNER[reps] = (fn, in_names, out_names, out_avals, zero_outs)
    return _RUNNER[reps]


def _global_inputs(x, Wk, Wq, Wv):
    """Concatenated per-core inputs keyed by BIR input name."""
    reps = {
        "x": np.ascontiguousarray(x, dtype=np.float32),
        "Wq": np.tile(np.asarray(Wq, np.float32), (N_CORES, 1)),
        "Wk": np.tile(np.asarray(Wk, np.float32), (N_CORES, 1)),
        "Wv": np.tile(np.asarray(Wv, np.float32), (N_CORES, 1)),
    }
    return reps


def kernel(x, Wk, Wq, Wv):
    x = np.asarray(x, dtype=np.float32)
    fn, in_names, out_names, out_avals, zero_outs = _get_runner()
    gi = _global_inputs(x, Wk, Wq, Wv)
    args = [gi[n] for n in in_names] + zero_outs
    outs = fn(*args)
    out = np.asarray(outs[out_names.index("out")])
    return out.astype(np.float32)
